# revision 25
# baseline (speedup 1.0000x reference)
"""CSAA (criss-cross axial attention) Trainium2 kernel, v2.

Sharding: pure data parallel — batch element b -> NeuronCore b (B=8 on 8 cores).

Per-core pipeline (R=128, CIN=COUT=256, H=W=128, HW=16384), per batch element:
  xT [c, (w,h)] bf16 (host-transposed)
    --stage A (per-w stationary mm)-->  Zp [h, (w,r)] bf16          (+br)
  width branch:  qkv (per-r stationary mm) -> qkvs [w, s,o,c]       (+corr_w)
                 S^T-form attention -> Wp [w, (h,r)] bf16
  height branch: qkv -> qkvs [h, s,o',c]                            (+corr_h)
                 S^T-form attention -> H2 [r, (w,h)] streamed
  restore:       Y [co, (w,h)] bf16 -> host f32 + transpose + bo

Attention (S^T form, softmax over the PARTITION axis):
  S^T[u,r] = matmul(lhsT=k_blk, rhs=q_blk); E = exp(S^T) bf16
  width:  sums = gpsimd.partition_all_reduce(E) (replicated f32);
          OutT[w,r] = matmul(lhsT=V^T_blk, rhs=E_blk); out = OutT / sums
  height: sums_T[r,1] = matmul(lhsT=E_blk, rhs=ones); rcT = 1/sums_T;
          Out[r,h] = matmul(lhsT=E_blk, rhs=V^T_blk); out = Out * rcT[r]

Bias exactness: q-bias via corr tile (affects softmax); k-bias dropped
(u-independent shift, softmax-invariant in S^T form); v-bias added at the
qkv copy via corr v-section (rides through attention exactly since softmax
rows sum to 1); bo added on host.
"""

import numpy as np
from contextlib import ExitStack

import ml_dtypes

R = 128
CIN = 256
COUT = 256
HW = R * R
NCORES = 8
LINEARIZE = False
_CACHE = {}


def _build():
    try:
        import concourse.bass as bass
    except ImportError:
        import sys
        for p in ("/opt/trn_rl_repo", "/root/.axon_site/_ro/trn_rl_repo"):
            if p not in sys.path:
                sys.path.append(p)
        import concourse.bass as bass
    import concourse.tile as tile
    from concourse import bacc, mybir, bass_isa

    BF = mybir.dt.bfloat16
    F32 = mybir.dt.float32
    AF = mybir.ActivationFunctionType
    ALU = mybir.AluOpType
    RED = bass_isa.ReduceOp
    ts = bass.ts

    nc = bacc.Bacc("TRN2", target_bir_lowering=False, debug=False)

    def din(name, shape, dt):
        return nc.dram_tensor(name, shape, dt, kind="ExternalInput").ap()

    xT = din("xT", [CIN, HW], BF)        # [c, w*128+h]
    wrT = din("wrT", [CIN, R], BF)
    brp_row = din("brp_row", [1, 512], BF)   # br[r] tiled x4, one partition
    ones_row = din("ones_row", [1, R], BF)
    wqkv_w = din("wqkv_w", [R, 3 * R], BF)   # [WqwT|WkwT|WvwT]
    corr_w = din("corr_w", [R, 2 * R], BF)   # [bqw tile | bvw tile]
    wqkv_h = din("wqkv_h", [R, 3 * R], BF)
    corr_h = din("corr_h", [R, 2 * R], BF)   # [bqh tile | bvh tile]
    woT = din("woT", [R, COUT], BF)
    ident = din("ident", [R, R], BF)
    ones = din("ones", [R, 1], BF)
    y = nc.dram_tensor("y", [COUT, HW], BF, kind="ExternalOutput").ap()

    OQ, OK, OV = 0, HW, 2 * HW

    with tile.TileContext(nc, linearize=LINEARIZE) as tc, ExitStack() as ctx:
        const = ctx.enter_context(tc.tile_pool(name="const", bufs=1))

        _cn = [0]

        def cload(ap, dt):
            _cn[0] += 1
            t = const.tile(list(ap.shape), dt, tag=f"c{_cn[0]}_{ap.tensor.name}")
            nc.sync.dma_start(t[:], ap)
            return t

        wrT_a = cload(wrT[0:R, :], BF)
        wrT_b = cload(wrT[R:CIN, :], BF)
        brp_sb = cload(brp_row, BF)
        ones_row_sb = cload(ones_row, BF)
        wqkvw_sb = cload(wqkv_w, BF)
        corrw_sb = cload(corr_w, BF)
        wqkvh_sb = cload(wqkv_h, BF)
        corrh_sb = cload(corr_h, BF)
        woT_sb = cload(woT, BF)
        id_sb = cload(ident, BF)
        ones_sb = cload(ones, BF)

        _rc = [0]

        def rot_copy(dst, src, seq="vs"):
            e = seq[_rc[0] % len(seq)]
            _rc[0] += 1
            if e == "v":
                nc.vector.tensor_copy(dst, src)
            else:
                nc.scalar.activation(dst, src, AF.Identity)

        qkvp = ctx.enter_context(tc.tile_pool(name="qkvp", bufs=1))
        qkvs = qkvp.tile([R, 3 * HW], BF, tag="qkvs")  # [q|k|v], [p, o*128+c]
        qkvs_c = qkvs[:].rearrange("p (s o c) -> p c s o", s=3, c=R)

        def qkv_phase(src, wq_sb, corr_sb):
            # src [p, (t, c)] : contract over p, per-c stationary slice
            src_v = src[:].rearrange("p (t c) -> p c t", c=R)
            corr_q = corr_sb[:, 0:R][:, None, :].broadcast_to([R, 4, R])
            corr_v = corr_sb[:, R:2 * R][:, None, :].broadcast_to([R, 4, R])
            with tc.tile_pool(name="pQ", bufs=2, space="PSUM") as pQ:
                for cg in range(32):
                    pq = pQ.tile([R, 2048], F32, tag="pq")
                    for i in range(4):
                        c = cg * 4 + i
                        nc.tensor.matmul(pq[:, i * 512:i * 512 + 384],
                                         src_v[:, c, :], wq_sb[:],
                                         start=True, stop=True)
                    pq_v = pq[:].rearrange("p (c x) -> p c x", c=4)
                    pq_v = pq_v[:, :, 0:384].rearrange("p c (s o) -> p c s o",
                                                       s=3)
                    cg4 = slice(cg * 4, cg * 4 + 4)
                    dq, dk, dv = (qkvs_c[:, cg4, s, :] for s in range(3))
                    sq, sk, sv = (pq_v[:, :, s, :] for s in range(3))
                    # k: pure copy on ACT; q,v: corr adds spread over
                    # DVE (direct psum add) / ACT-copy + Pool in-place add
                    nc.scalar.activation(dk, sk, AF.Identity)
                    m = cg % 4
                    if m == 1:
                        nc.scalar.activation(dq, sq, AF.Identity)
                        nc.gpsimd.tensor_tensor(dq, dq, corr_q, op=ALU.add)
                        nc.vector.tensor_tensor(dv, sv, corr_v, op=ALU.add)
                    elif m == 3:
                        nc.vector.tensor_tensor(dq, sq, corr_q, op=ALU.add)
                        nc.scalar.activation(dv, sv, AF.Identity)
                        nc.gpsimd.tensor_tensor(dv, dv, corr_v, op=ALU.add)
                    else:
                        nc.vector.tensor_tensor(dq, sq, corr_q, op=ALU.add)
                        nc.vector.tensor_tensor(dv, sv, corr_v, op=ALU.add)

        def attn_w_phase(wp):
            with tc.tile_pool(name="aw", bufs=4) as aw, \
                 tc.tile_pool(name="pmm", bufs=3, space="PSUM") as pmm, \
                 tc.tile_pool(name="pvt", bufs=2, space="PSUM") as pvt:
                for og in range(32):
                    pS = pmm.tile([R, 512], F32, tag="ps")
                    for i in range(4):
                        o = og * 4 + i
                        nc.tensor.matmul(pS[:, ts(i, R)],
                                         qkvs[:, OK + o * R:OK + (o + 1) * R],
                                         qkvs[:, OQ + o * R:OQ + (o + 1) * R],
                                         start=True, stop=True)
                    e4 = aw.tile([R, 512], BF, tag="e4")
                    nc.scalar.activation(e4[:], pS[:], AF.Exp)
                    sums = aw.tile([R, 512], F32, tag="sums")
                    nc.gpsimd.partition_all_reduce(sums[:], e4[:], R, RED.add)
                    rc = aw.tile([R, 512], BF, tag="rc")
                    with nc.allow_low_precision(reason="softmax recip bf16"):
                        nc.vector.reciprocal(rc[:], sums[:])
                    en = aw.tile([R, 512], BF, tag="en")
                    nc.vector.tensor_tensor(en[:], e4[:], rc[:], op=ALU.mult)
                    pvT = pvt.tile([R, 512], BF, tag="tp")
                    for i in range(4):
                        o = og * 4 + i
                        nc.tensor.transpose(pvT[:, ts(i, R)],
                                            qkvs[:, OV + o * R:OV + (o + 1) * R],
                                            id_sb[:])
                    vts = aw.tile([R, 512], BF, tag="vts")
                    rot_copy(vts[:], pvT[:], seq="vs")
                    pO = pmm.tile([R, 512], F32, tag="po")
                    for i in range(4):
                        nc.tensor.matmul(pO[:, ts(i, R)], vts[:, ts(i, R)],
                                         en[:, ts(i, R)], start=True, stop=True)
                    nc.scalar.activation(wp[:, ts(og, 512)], pO[:],
                                         AF.Identity)

        def attn_h_restore(ybf_pool, h2_pool):
            with tc.tile_pool(name="ah", bufs=4) as ah, \
                 tc.tile_pool(name="pmm", bufs=2, space="PSUM") as pmm, \
                 tc.tile_pool(name="pvt", bufs=2, space="PSUM") as pvt:
                for og in range(32):
                    pS = pmm.tile([R, 512], F32, tag="ps")
                    for i in range(4):
                        o = og * 4 + i
                        nc.tensor.matmul(pS[:, ts(i, R)],
                                         qkvs[:, OK + o * R:OK + (o + 1) * R],
                                         qkvs[:, OQ + o * R:OQ + (o + 1) * R],
                                         start=True, stop=True)
                    e4 = ah.tile([R, 512], BF, tag="e4")
                    nc.scalar.activation(e4[:], pS[:], AF.Exp)
                    pvT = pvt.tile([R, 512], BF, tag="tp")
                    for i in range(4):
                        o = og * 4 + i
                        nc.tensor.transpose(pvT[:, ts(i, R)],
                                            qkvs[:, OV + o * R:OV + (o + 1) * R],
                                            id_sb[:])
                    vts = ah.tile([R, 512], BF, tag="vts")
                    rot_copy(vts[:], pvT[:], seq="vs")
                    # column sums: reuse pS (exp already consumed it)
                    for i in range(4):
                        nc.tensor.matmul(pS[:, i:i + 1], e4[:, ts(i, R)],
                                         ones_sb[:], start=True, stop=True)
                    rcT = ah.tile([R, 4], F32, tag="rcT")
                    nc.vector.reciprocal(rcT[:], pS[:, 0:4])
                    pO = pmm.tile([R, 512], F32, tag="po")
                    for i in range(4):
                        nc.tensor.matmul(pO[:, ts(i, R)], e4[:, ts(i, R)],
                                         vts[:, ts(i, R)], start=True,
                                         stop=True)
                    h2 = h2_pool.tile([R, 512], BF, tag="h2")
                    for i in range(4):
                        if i % 2 == 0:
                            nc.scalar.activation(h2[:, ts(i, R)],
                                                 pO[:, ts(i, R)], AF.Identity,
                                                 scale=rcT[:, i:i + 1])
                        else:
                            nc.vector.tensor_scalar_mul(
                                h2[:, ts(i, R)], pO[:, ts(i, R)],
                                rcT[:, i:i + 1])
                    # restore: y chunk for this og (bo added on host)
                    for half in range(2):
                        pY = pmm.tile([R, 512], F32, tag="py")
                        nc.tensor.matmul(pY[:], woT_sb[:, ts(half, R)],
                                         h2[:], start=True, stop=True)
                        yt = ybf_pool.tile([R, 512], BF, tag="yt")
                        rot_copy(yt[:], pY[:], seq="svg")
                        nc.sync.dma_start(y[half * R:(half + 1) * R,
                                            ts(og, 512)], yt[:])

        # ---------- Stage A + width qkv (zp scoped) ----------
        with tc.tile_pool(name="pAB", bufs=1) as pAB:
            zp = pAB.tile([R, HW], BF, tag="zp")  # [h, w*128+r]
            with tc.tile_pool(name="xin", bufs=4) as xin, \
                 tc.tile_pool(name="pA", bufs=3, space="PSUM") as pA:
                for ch in range(8):          # 16 w-blocks per chunk
                    xa = xin.tile([R, 2048], BF, tag="xa")
                    nc.sync.dma_start(xa[:], xT[0:R, ts(ch, 2048)])
                    xb = xin.tile([R, 2048], BF, tag="xb")
                    nc.sync.dma_start(xb[:], xT[R:CIN, ts(ch, 2048)])
                    for wg in range(4):      # 4 w per psum tile
                        pa = pA.tile([R, 512], F32, tag="pa")
                        for i in range(4):
                            wl = wg * 4 + i  # w within chunk
                            nc.tensor.matmul(pa[:, ts(i, R)],
                                             xa[:, ts(wl, R)], wrT_a[:],
                                             start=True, stop=False)
                            nc.tensor.matmul(pa[:, ts(i, R)],
                                             xb[:, ts(wl, R)], wrT_b[:],
                                             start=False, stop=False)
                            # + br via rank-1 accumulate, closes the group
                            nc.tensor.matmul(pa[:, ts(i, R)], ones_row_sb[:],
                                             brp_sb[:, 0:R],
                                             start=False, stop=True)
                        g = ch * 4 + wg
                        rot_copy(zp[:, ts(g, 512)], pa[:], seq="vs")

            qkv_phase(zp, wqkvw_sb, corrw_sb)

        # ---------- width attention -> wp, height qkv ----------
        with tc.tile_pool(name="pW", bufs=1) as pW:
            wp = pW.tile([R, HW], BF, tag="wp")  # [w, o*128+r]
            attn_w_phase(wp)
            qkv_phase(wp, wqkvh_sb, corrh_sb)

        # ---------- height attention + restore (streamed) ----------
        with tc.tile_pool(name="h2p", bufs=4) as h2p, \
             tc.tile_pool(name="yout", bufs=6) as yp:
            attn_h_restore(yp, h2p)

    nc.compile()
    return nc


def _get_nc():
    if "nc" not in _CACHE:
        _CACHE["nc"] = _build()
    return _CACHE["nc"]


def _host_inputs(inputs):
    bf16 = ml_dtypes.bfloat16
    f32 = np.float32
    g = {k: np.asarray(v) for k, v in inputs.items()}
    Wr, br = g["Wr"], g["br"]
    Wqw, bqw, Wkw, Wvw, bvw = g["Wqw"], g["bqw"], g["Wkw"], g["Wvw"], g["bvw"]
    Wqh, bqh, Wkh, Wvh, bvh = g["Wqh"], g["bqh"], g["Wkh"], g["Wvh"], g["bvh"]
    Wo = g["Wo"]

    def tile_row(v):
        return np.tile(np.asarray(v)[None, :], (R, 1))

    com = dict(
        wrT=np.ascontiguousarray(Wr.T).astype(bf16),
        brp_row=np.ascontiguousarray(np.tile(np.asarray(br)[None, :], (1, 4))
                                     ).astype(bf16),
        ones_row=np.ones((1, R)).astype(bf16),
        wqkv_w=np.ascontiguousarray(
            np.concatenate([Wqw.T, Wkw.T, Wvw.T], 1)).astype(bf16),
        corr_w=np.ascontiguousarray(np.concatenate(
            [tile_row(bqw), tile_row(bvw)], 1)).astype(bf16),
        wqkv_h=np.ascontiguousarray(
            np.concatenate([Wqh.T, Wkh.T, Wvh.T], 1)).astype(bf16),
        corr_h=np.ascontiguousarray(np.concatenate(
            [tile_row(bqh), tile_row(bvh)], 1)).astype(bf16),
        woT=np.ascontiguousarray(Wo.T).astype(bf16),
        ident=np.eye(R).astype(bf16),
        ones=np.ones((R, 1)).astype(bf16),
    )
    xs = np.asarray(g["x"])
    in_maps = []
    for b in range(NCORES):
        m = dict(com)
        m["xT"] = np.ascontiguousarray(
            xs[b].transpose(0, 2, 1).reshape(CIN, HW)).astype(bf16)
        in_maps.append(m)
    return in_maps


def run(inputs, trace=False, **kw):
    try:
        from concourse.bass_utils import run_bass_kernel_spmd
    except ImportError:
        import sys
        for p in ("/opt/trn_rl_repo", "/root/.axon_site/_ro/trn_rl_repo"):
            if p not in sys.path:
                sys.path.append(p)
        from concourse.bass_utils import run_bass_kernel_spmd
    nc = _get_nc()
    in_maps = _host_inputs(inputs)
    res = run_bass_kernel_spmd(nc, in_maps, list(range(NCORES)),
                               trace=trace, **kw)
    bo = np.asarray(inputs["bo"], dtype=np.float32)
    out = np.stack([np.asarray(res.results[b]["y"], dtype=np.float32)
                    .reshape(COUT, R, R).transpose(0, 2, 1)
                    for b in range(NCORES)], 0)
    out += bo[None, :, None, None]
    return out.astype(np.float32), res


def kernel(**inputs):
    out, _ = run(inputs, trace=False)
    return out


# revision 27
# speedup vs baseline: 1.1074x; 1.1074x over previous
"""CSAA (criss-cross axial attention) Trainium2 kernel, v2.

Sharding: pure data parallel — batch element b -> NeuronCore b (B=8 on 8 cores).

Per-core pipeline (R=128, CIN=COUT=256, H=W=128, HW=16384), per batch element:
  xT [c, (w,h)] bf16 (host-transposed)
    --stage A (per-w stationary mm)-->  Zp [h, (w,r)] bf16          (+br)
  width branch:  qkv (per-r stationary mm) -> qkvs [w, s,o,c]       (+corr_w)
                 S^T-form attention -> Wp [w, (h,r)] bf16
  height branch: qkv -> qkvs [h, s,o',c]                            (+corr_h)
                 S^T-form attention -> H2 [r, (w,h)] streamed
  restore:       Y [co, (w,h)] bf16 -> host f32 + transpose + bo

Attention (S^T form, softmax over the PARTITION axis):
  S^T[u,r] = matmul(lhsT=k_blk, rhs=q_blk); E = exp(S^T) bf16
  width:  sums = gpsimd.partition_all_reduce(E) (replicated f32);
          OutT[w,r] = matmul(lhsT=V^T_blk, rhs=E_blk); out = OutT / sums
  height: sums_T[r,1] = matmul(lhsT=E_blk, rhs=ones); rcT = 1/sums_T;
          Out[r,h] = matmul(lhsT=E_blk, rhs=V^T_blk); out = Out * rcT[r]

Bias exactness: q-bias via corr tile (affects softmax); k-bias dropped
(u-independent shift, softmax-invariant in S^T form); v-bias added at the
qkv copy via corr v-section (rides through attention exactly since softmax
rows sum to 1); bo added on host.
"""

import numpy as np
from contextlib import ExitStack

import ml_dtypes

R = 128
CIN = 256
COUT = 256
HW = R * R
NCORES = 8
LINEARIZE = False
_CACHE = {}


def _build():
    try:
        import concourse.bass as bass
    except ImportError:
        import sys
        for p in ("/opt/trn_rl_repo", "/root/.axon_site/_ro/trn_rl_repo"):
            if p not in sys.path:
                sys.path.append(p)
        import concourse.bass as bass
    import concourse.tile as tile
    from concourse import bacc, mybir, bass_isa

    BF = mybir.dt.bfloat16
    F32 = mybir.dt.float32
    AF = mybir.ActivationFunctionType
    ALU = mybir.AluOpType
    RED = bass_isa.ReduceOp
    ts = bass.ts

    nc = bacc.Bacc("TRN2", target_bir_lowering=False, debug=False)

    def din(name, shape, dt):
        return nc.dram_tensor(name, shape, dt, kind="ExternalInput").ap()

    xT = din("xT", [CIN, HW], BF)        # [c, w*128+h]
    wrT = din("wrT", [CIN, R], BF)
    brp_row = din("brp_row", [1, 512], BF)   # br[r] tiled x4, one partition
    ones_row = din("ones_row", [1, R], BF)
    wqkv_w = din("wqkv_w", [R, 3 * R], BF)   # [WqwT|WkwT|WvwT]
    corr_w = din("corr_w", [R, 2 * R], BF)   # [bqw tile | bvw tile]
    wqkv_h = din("wqkv_h", [R, 3 * R], BF)
    corr_h = din("corr_h", [R, 2 * R], BF)   # [bqh tile | bvh tile]
    woT = din("woT", [R, COUT], BF)
    ident = din("ident", [R, R], BF)
    ones = din("ones", [R, 1], BF)
    y = nc.dram_tensor("y", [COUT, HW], BF, kind="ExternalOutput").ap()

    OQ, OK, OV = 0, HW, 2 * HW

    with tile.TileContext(nc, linearize=LINEARIZE) as tc, ExitStack() as ctx:
        const = ctx.enter_context(tc.tile_pool(name="const", bufs=1))

        _cn = [0]

        def cload(ap, dt):
            _cn[0] += 1
            t = const.tile(list(ap.shape), dt, tag=f"c{_cn[0]}_{ap.tensor.name}")
            nc.sync.dma_start(t[:], ap)
            return t

        wrT_a = cload(wrT[0:R, :], BF)
        wrT_b = cload(wrT[R:CIN, :], BF)
        brp_sb = cload(brp_row, BF)
        ones_row_sb = cload(ones_row, BF)
        wqkvw_sb = cload(wqkv_w, BF)
        corrw_sb = cload(corr_w, BF)
        wqkvh_sb = cload(wqkv_h, BF)
        corrh_sb = cload(corr_h, BF)
        woT_sb = cload(woT, BF)
        id_sb = cload(ident, BF)
        ones_sb = cload(ones, BF)

        _rc = [0]

        def rot_copy(dst, src, seq="vs"):
            e = seq[_rc[0] % len(seq)]
            _rc[0] += 1
            if e == "v":
                nc.vector.tensor_copy(dst, src)
            else:
                nc.scalar.activation(dst, src, AF.Identity)

        qkvp = ctx.enter_context(tc.tile_pool(name="qkvp", bufs=1))
        qkvs = qkvp.tile([R, 3 * HW], BF, tag="qkvs")  # [q|k|v], [p, o*128+c]
        qkvs_c = qkvs[:].rearrange("p (s o c) -> p c s o", s=3, c=R)

        def qkv_phase(src, wq_sb, corr_sb):
            # src [p, (t, c)] : contract over p, per-c stationary slice
            src_v = src[:].rearrange("p (t c) -> p c t", c=R)
            corr_q = corr_sb[:, 0:R][:, None, :].broadcast_to([R, 4, R])
            corr_v = corr_sb[:, R:2 * R][:, None, :].broadcast_to([R, 4, R])
            with tc.tile_pool(name="pQ", bufs=2, space="PSUM") as pQ:
                for cg in range(32):
                    pq = pQ.tile([R, 2048], F32, tag="pq")
                    for i in range(4):
                        c = cg * 4 + i
                        nc.tensor.matmul(pq[:, i * 512:i * 512 + 384],
                                         src_v[:, c, :], wq_sb[:],
                                         start=True, stop=True)
                    pq_v = pq[:].rearrange("p (c x) -> p c x", c=4)
                    pq_v = pq_v[:, :, 0:384].rearrange("p c (s o) -> p c s o",
                                                       s=3)
                    cg4 = slice(cg * 4, cg * 4 + 4)
                    dq, dk, dv = (qkvs_c[:, cg4, s, :] for s in range(3))
                    sq, sk, sv = (pq_v[:, :, s, :] for s in range(3))
                    # k: pure copy on ACT; q,v: corr adds spread over
                    # DVE (direct psum add) / ACT-copy + Pool in-place add
                    nc.scalar.activation(dk, sk, AF.Identity)
                    m = cg % 4
                    if m == 1:
                        nc.scalar.activation(dq, sq, AF.Identity)
                        nc.gpsimd.tensor_tensor(dq, dq, corr_q, op=ALU.add)
                        nc.vector.tensor_tensor(dv, sv, corr_v, op=ALU.add)
                    elif m == 3:
                        nc.vector.tensor_tensor(dq, sq, corr_q, op=ALU.add)
                        nc.scalar.activation(dv, sv, AF.Identity)
                        nc.gpsimd.tensor_tensor(dv, dv, corr_v, op=ALU.add)
                    else:
                        nc.vector.tensor_tensor(dq, sq, corr_q, op=ALU.add)
                        nc.vector.tensor_tensor(dv, sv, corr_v, op=ALU.add)

        def attn_w_phase(wp):
            with tc.tile_pool(name="aw", bufs=4) as aw, \
                 tc.tile_pool(name="pmm", bufs=3, space="PSUM") as pmm, \
                 tc.tile_pool(name="pvt", bufs=2, space="PSUM") as pvt:
                for og in range(32):
                    pS = pmm.tile([R, 512], F32, tag="ps")
                    for i in range(4):
                        o = og * 4 + i
                        nc.tensor.matmul(pS[:, ts(i, R)],
                                         qkvs[:, OK + o * R:OK + (o + 1) * R],
                                         qkvs[:, OQ + o * R:OQ + (o + 1) * R],
                                         start=True, stop=True)
                    e4 = aw.tile([R, 512], BF, tag="e4")
                    nc.scalar.activation(e4[:], pS[:], AF.Exp)
                    sums = aw.tile([R, 512], F32, tag="sums")
                    nc.gpsimd.partition_all_reduce(sums[:], e4[:], R, RED.add)
                    rc = aw.tile([R, 512], BF, tag="rc")
                    with nc.allow_low_precision(reason="softmax recip bf16"):
                        nc.vector.reciprocal(rc[:], sums[:])
                    en = aw.tile([R, 512], BF, tag="en")
                    nc.vector.tensor_tensor(en[:], e4[:], rc[:], op=ALU.mult)
                    pvT = pvt.tile([R, 512], BF, tag="tp")
                    for i in range(4):
                        o = og * 4 + i
                        nc.tensor.transpose(pvT[:, ts(i, R)],
                                            qkvs[:, OV + o * R:OV + (o + 1) * R],
                                            id_sb[:])
                    vts = aw.tile([R, 512], BF, tag="vts")
                    nc.vector.tensor_copy(vts[:], pvT[:])
                    pO = pmm.tile([R, 512], F32, tag="po")
                    for i in range(4):
                        nc.tensor.matmul(pO[:, ts(i, R)], vts[:, ts(i, R)],
                                         en[:, ts(i, R)], start=True, stop=True)
                    rot_copy(wp[:, ts(og, 512)], pO[:], seq="vss")

        def attn_h_restore(ybf_pool, h2_pool):
            with tc.tile_pool(name="ah", bufs=4) as ah, \
                 tc.tile_pool(name="pmm", bufs=2, space="PSUM") as pmm, \
                 tc.tile_pool(name="pvt", bufs=2, space="PSUM") as pvt:
                for og in range(32):
                    pS = pmm.tile([R, 512], F32, tag="ps")
                    for i in range(4):
                        o = og * 4 + i
                        nc.tensor.matmul(pS[:, ts(i, R)],
                                         qkvs[:, OK + o * R:OK + (o + 1) * R],
                                         qkvs[:, OQ + o * R:OQ + (o + 1) * R],
                                         start=True, stop=True)
                    e4 = ah.tile([R, 512], BF, tag="e4")
                    nc.scalar.activation(e4[:], pS[:], AF.Exp)
                    pvT = pvt.tile([R, 512], BF, tag="tp")
                    for i in range(4):
                        o = og * 4 + i
                        nc.tensor.transpose(pvT[:, ts(i, R)],
                                            qkvs[:, OV + o * R:OV + (o + 1) * R],
                                            id_sb[:])
                    vts = ah.tile([R, 512], BF, tag="vts")
                    rot_copy(vts[:], pvT[:], seq="vs")
                    # column sums: reuse pS (exp already consumed it)
                    for i in range(4):
                        nc.tensor.matmul(pS[:, i:i + 1], e4[:, ts(i, R)],
                                         ones_sb[:], start=True, stop=True)
                    rcT = ah.tile([R, 4], F32, tag="rcT")
                    nc.vector.reciprocal(rcT[:], pS[:, 0:4])
                    pO = pmm.tile([R, 512], F32, tag="po")
                    for i in range(4):
                        nc.tensor.matmul(pO[:, ts(i, R)], e4[:, ts(i, R)],
                                         vts[:, ts(i, R)], start=True,
                                         stop=True)
                    h2 = h2_pool.tile([R, 512], BF, tag="h2")
                    for i in range(4):
                        if i % 4 == 0:
                            nc.scalar.activation(h2[:, ts(i, R)],
                                                 pO[:, ts(i, R)], AF.Identity,
                                                 scale=rcT[:, i:i + 1])
                        else:
                            nc.vector.tensor_scalar_mul(
                                h2[:, ts(i, R)], pO[:, ts(i, R)],
                                rcT[:, i:i + 1])
                    # restore: y chunk for this og (bo added on host)
                    for half in range(2):
                        pY = pmm.tile([R, 512], F32, tag="py")
                        nc.tensor.matmul(pY[:], woT_sb[:, ts(half, R)],
                                         h2[:], start=True, stop=True)
                        yt = ybf_pool.tile([R, 512], BF, tag="yt")
                        rot_copy(yt[:], pY[:], seq="sv")
                        nc.sync.dma_start(y[half * R:(half + 1) * R,
                                            ts(og, 512)], yt[:])

        # ---------- Stage A + width qkv (zp scoped) ----------
        with tc.tile_pool(name="pAB", bufs=1) as pAB:
            zp = pAB.tile([R, HW], BF, tag="zp")  # [h, w*128+r]
            with tc.tile_pool(name="xin", bufs=4) as xin, \
                 tc.tile_pool(name="pA", bufs=3, space="PSUM") as pA:
                for ch in range(8):          # 16 w-blocks per chunk
                    xa = xin.tile([R, 2048], BF, tag="xa")
                    nc.sync.dma_start(xa[:], xT[0:R, ts(ch, 2048)])
                    xb = xin.tile([R, 2048], BF, tag="xb")
                    nc.sync.dma_start(xb[:], xT[R:CIN, ts(ch, 2048)])
                    for wg in range(4):      # 4 w per psum tile
                        pa = pA.tile([R, 512], F32, tag="pa")
                        for i in range(4):
                            wl = wg * 4 + i  # w within chunk
                            nc.tensor.matmul(pa[:, ts(i, R)],
                                             xa[:, ts(wl, R)], wrT_a[:],
                                             start=True, stop=False)
                            nc.tensor.matmul(pa[:, ts(i, R)],
                                             xb[:, ts(wl, R)], wrT_b[:],
                                             start=False, stop=False)
                            # + br via rank-1 accumulate, closes the group
                            nc.tensor.matmul(pa[:, ts(i, R)], ones_row_sb[:],
                                             brp_sb[:, 0:R],
                                             start=False, stop=True)
                        g = ch * 4 + wg
                        rot_copy(zp[:, ts(g, 512)], pa[:], seq="vs")

            qkv_phase(zp, wqkvw_sb, corrw_sb)

        # ---------- width attention -> wp, height qkv ----------
        with tc.tile_pool(name="pW", bufs=1) as pW:
            wp = pW.tile([R, HW], BF, tag="wp")  # [w, o*128+r]
            attn_w_phase(wp)
            qkv_phase(wp, wqkvh_sb, corrh_sb)

        # ---------- height attention + restore (streamed) ----------
        with tc.tile_pool(name="h2p", bufs=4) as h2p, \
             tc.tile_pool(name="yout", bufs=6) as yp:
            attn_h_restore(yp, h2p)

    nc.compile()
    return nc


def _get_nc():
    if "nc" not in _CACHE:
        _CACHE["nc"] = _build()
    return _CACHE["nc"]


def _host_inputs(inputs):
    bf16 = ml_dtypes.bfloat16
    f32 = np.float32
    g = {k: np.asarray(v) for k, v in inputs.items()}
    Wr, br = g["Wr"], g["br"]
    Wqw, bqw, Wkw, Wvw, bvw = g["Wqw"], g["bqw"], g["Wkw"], g["Wvw"], g["bvw"]
    Wqh, bqh, Wkh, Wvh, bvh = g["Wqh"], g["bqh"], g["Wkh"], g["Wvh"], g["bvh"]
    Wo = g["Wo"]

    def tile_row(v):
        return np.tile(np.asarray(v)[None, :], (R, 1))

    com = dict(
        wrT=np.ascontiguousarray(Wr.T).astype(bf16),
        brp_row=np.ascontiguousarray(np.tile(np.asarray(br)[None, :], (1, 4))
                                     ).astype(bf16),
        ones_row=np.ones((1, R)).astype(bf16),
        wqkv_w=np.ascontiguousarray(
            np.concatenate([Wqw.T, Wkw.T, Wvw.T], 1)).astype(bf16),
        corr_w=np.ascontiguousarray(np.concatenate(
            [tile_row(bqw), tile_row(bvw)], 1)).astype(bf16),
        wqkv_h=np.ascontiguousarray(
            np.concatenate([Wqh.T, Wkh.T, Wvh.T], 1)).astype(bf16),
        corr_h=np.ascontiguousarray(np.concatenate(
            [tile_row(bqh), tile_row(bvh)], 1)).astype(bf16),
        woT=np.ascontiguousarray(Wo.T).astype(bf16),
        ident=np.eye(R).astype(bf16),
        ones=np.ones((R, 1)).astype(bf16),
    )
    xs = np.asarray(g["x"])
    in_maps = []
    for b in range(NCORES):
        m = dict(com)
        m["xT"] = np.ascontiguousarray(
            xs[b].transpose(0, 2, 1).reshape(CIN, HW)).astype(bf16)
        in_maps.append(m)
    return in_maps


def run(inputs, trace=False, **kw):
    try:
        from concourse.bass_utils import run_bass_kernel_spmd
    except ImportError:
        import sys
        for p in ("/opt/trn_rl_repo", "/root/.axon_site/_ro/trn_rl_repo"):
            if p not in sys.path:
                sys.path.append(p)
        from concourse.bass_utils import run_bass_kernel_spmd
    nc = _get_nc()
    in_maps = _host_inputs(inputs)
    res = run_bass_kernel_spmd(nc, in_maps, list(range(NCORES)),
                               trace=trace, **kw)
    bo = np.asarray(inputs["bo"], dtype=np.float32)
    out = np.stack([np.asarray(res.results[b]["y"], dtype=np.float32)
                    .reshape(COUT, R, R).transpose(0, 2, 1)
                    for b in range(NCORES)], 0)
    out += bo[None, :, None, None]
    return out.astype(np.float32), res


def kernel(**inputs):
    out, _ = run(inputs, trace=False)
    return out


# revision 32
# speedup vs baseline: 1.1281x; 1.0187x over previous
"""CSAA (criss-cross axial attention) Trainium2 kernel, v2.

Sharding: pure data parallel — batch element b -> NeuronCore b (B=8 on 8 cores).

Per-core pipeline (R=128, CIN=COUT=256, H=W=128, HW=16384), per batch element:
  xT [c, (w,h)] bf16 (host-transposed)
    --stage A (per-w stationary mm)-->  Zp [h, (w,r)] bf16          (+br)
  width branch:  qkv (per-r stationary mm) -> qkvs [w, s,o,c]       (+corr_w)
                 S^T-form attention -> Wp [w, (h,r)] bf16
  height branch: qkv -> qkvs [h, s,o',c]                            (+corr_h)
                 S^T-form attention -> H2 [r, (w,h)] streamed
  restore:       Y [co, (w,h)] bf16 -> host f32 + transpose + bo

Attention (S^T form, softmax over the PARTITION axis):
  S^T[u,r] = matmul(lhsT=k_blk, rhs=q_blk); E = exp(S^T) bf16
  width:  sums = gpsimd.partition_all_reduce(E) (replicated f32);
          OutT[w,r] = matmul(lhsT=V^T_blk, rhs=E_blk); out = OutT / sums
  height: sums_T[r,1] = matmul(lhsT=E_blk, rhs=ones); rcT = 1/sums_T;
          Out[r,h] = matmul(lhsT=E_blk, rhs=V^T_blk); out = Out * rcT[r]

Bias exactness: q-bias via corr tile (affects softmax); k-bias dropped
(u-independent shift, softmax-invariant in S^T form); v-bias added at the
qkv copy via corr v-section (rides through attention exactly since softmax
rows sum to 1); bo added on host.
"""

import numpy as np
from contextlib import ExitStack

import ml_dtypes

R = 128
CIN = 256
COUT = 256
HW = R * R
NCORES = 8
LINEARIZE = False
_CACHE = {}


def _build():
    try:
        import concourse.bass as bass
    except ImportError:
        import sys
        for p in ("/opt/trn_rl_repo", "/root/.axon_site/_ro/trn_rl_repo"):
            if p not in sys.path:
                sys.path.append(p)
        import concourse.bass as bass
    import concourse.tile as tile
    from concourse import bacc, mybir, bass_isa

    BF = mybir.dt.bfloat16
    F32 = mybir.dt.float32
    AF = mybir.ActivationFunctionType
    ALU = mybir.AluOpType
    RED = bass_isa.ReduceOp
    ts = bass.ts

    nc = bacc.Bacc("TRN2", target_bir_lowering=False, debug=False)

    def din(name, shape, dt):
        return nc.dram_tensor(name, shape, dt, kind="ExternalInput").ap()

    xT = din("xT", [CIN, HW], BF)        # [c, w*128+h]
    wrT = din("wrT", [CIN, R], BF)
    brp = din("brp", [R, 512], F32)      # br[r] tiled x4, replicated rows
    wqkv_w = din("wqkv_w", [R, 3 * R], BF)   # [WqwT|WkwT|WvwT]
    corr_w = din("corr_w", [R, 2 * R], BF)   # [bqw tile | bvw tile]
    wqkv_h = din("wqkv_h", [R, 3 * R], BF)
    corr_h = din("corr_h", [R, 2 * R], BF)   # [bqh tile | bvh tile]
    woT = din("woT", [R, COUT], BF)
    ident = din("ident", [R, R], BF)
    ones = din("ones", [R, 1], BF)
    y = nc.dram_tensor("y", [COUT, HW], BF, kind="ExternalOutput").ap()

    OQ, OK, OV = 0, HW, 2 * HW

    with tile.TileContext(nc, linearize=LINEARIZE) as tc, ExitStack() as ctx:
        const = ctx.enter_context(tc.tile_pool(name="const", bufs=1))

        _cn = [0]

        def cload(ap, dt):
            _cn[0] += 1
            t = const.tile(list(ap.shape), dt, tag=f"c{_cn[0]}_{ap.tensor.name}")
            nc.sync.dma_start(t[:], ap)
            return t

        wrT_a = cload(wrT[0:R, :], BF)
        wrT_b = cload(wrT[R:CIN, :], BF)
        brp_sb = cload(brp, F32)
        wqkvw_sb = cload(wqkv_w, BF)
        corrw_sb = cload(corr_w, BF)
        wqkvh_sb = cload(wqkv_h, BF)
        corrh_sb = cload(corr_h, BF)
        woT_sb = cload(woT, BF)
        id_sb = cload(ident, BF)
        ones_sb = cload(ones, BF)

        _rc = [0]

        def rot_copy(dst, src, seq="vs"):
            e = seq[_rc[0] % len(seq)]
            _rc[0] += 1
            if e == "v":
                nc.vector.tensor_copy(dst, src)
            else:
                nc.scalar.activation(dst, src, AF.Identity)

        qkvp = ctx.enter_context(tc.tile_pool(name="qkvp", bufs=1))
        qkvs = qkvp.tile([R, 3 * HW], BF, tag="qkvs")  # [q|k|v], [p, o*128+c]
        qkvs_c = qkvs[:].rearrange("p (s o c) -> p c s o", s=3, c=R)

        def qkv_phase(src, wq_sb, corr_sb):
            # src [p, (t, c)] : contract over p, per-c stationary slice
            src_v = src[:].rearrange("p (t c) -> p c t", c=R)
            corr_q = corr_sb[:, 0:R][:, None, :].broadcast_to([R, 4, R])
            corr_v = corr_sb[:, R:2 * R][:, None, :].broadcast_to([R, 4, R])
            with tc.tile_pool(name="pQ", bufs=2, space="PSUM") as pQ:
                for cg in range(32):
                    pq = pQ.tile([R, 2048], F32, tag="pq")
                    for i in range(4):
                        c = cg * 4 + i
                        nc.tensor.matmul(pq[:, i * 512:i * 512 + 384],
                                         src_v[:, c, :], wq_sb[:],
                                         start=True, stop=True)
                    pq_v = pq[:].rearrange("p (c x) -> p c x", c=4)
                    pq_v = pq_v[:, :, 0:384].rearrange("p c (s o) -> p c s o",
                                                       s=3)
                    cg4 = slice(cg * 4, cg * 4 + 4)
                    dq, dk, dv = (qkvs_c[:, cg4, s, :] for s in range(3))
                    sq, sk, sv = (pq_v[:, :, s, :] for s in range(3))
                    # k: pure copy on ACT; q,v: corr adds spread over
                    # DVE (direct psum add) / ACT-copy + Pool in-place add
                    nc.scalar.activation(dk, sk, AF.Identity)
                    m = cg % 4
                    if m == 1:
                        nc.scalar.activation(dq, sq, AF.Identity)
                        nc.gpsimd.tensor_tensor(dq, dq, corr_q, op=ALU.add)
                        nc.vector.tensor_tensor(dv, sv, corr_v, op=ALU.add)
                    elif m == 3:
                        nc.vector.tensor_tensor(dq, sq, corr_q, op=ALU.add)
                        nc.scalar.activation(dv, sv, AF.Identity)
                        nc.gpsimd.tensor_tensor(dv, dv, corr_v, op=ALU.add)
                    else:
                        nc.vector.tensor_tensor(dq, sq, corr_q, op=ALU.add)
                        nc.vector.tensor_tensor(dv, sv, corr_v, op=ALU.add)

        def attn_w_phase(wp):
            with tc.tile_pool(name="aw", bufs=4) as aw, \
                 tc.tile_pool(name="pmm", bufs=3, space="PSUM") as pmm, \
                 tc.tile_pool(name="pvt", bufs=2, space="PSUM") as pvt:
                for og in range(32):
                    pS = pmm.tile([R, 512], F32, tag="ps")
                    for i in range(4):
                        o = og * 4 + i
                        nc.tensor.matmul(pS[:, ts(i, R)],
                                         qkvs[:, OK + o * R:OK + (o + 1) * R],
                                         qkvs[:, OQ + o * R:OQ + (o + 1) * R],
                                         start=True, stop=True)
                    e4 = aw.tile([R, 512], BF, tag="e4")
                    nc.scalar.activation(e4[:], pS[:], AF.Exp)
                    sums = aw.tile([R, 512], F32, tag="sums")
                    nc.gpsimd.partition_all_reduce(sums[:], e4[:], R, RED.add)
                    rc = aw.tile([R, 512], BF, tag="rc")
                    with nc.allow_low_precision(reason="softmax recip bf16"):
                        nc.vector.reciprocal(rc[:], sums[:])
                    en = aw.tile([R, 512], BF, tag="en")
                    nc.vector.tensor_tensor(en[:], e4[:], rc[:], op=ALU.mult)
                    pvT = pvt.tile([R, 512], BF, tag="tp")
                    for i in range(4):
                        o = og * 4 + i
                        nc.tensor.transpose(pvT[:, ts(i, R)],
                                            qkvs[:, OV + o * R:OV + (o + 1) * R],
                                            id_sb[:])
                    vts = aw.tile([R, 512], BF, tag="vts")
                    nc.vector.tensor_copy(vts[:], pvT[:])
                    pO = pmm.tile([R, 512], F32, tag="po")
                    for i in range(4):
                        nc.tensor.matmul(pO[:, ts(i, R)], vts[:, ts(i, R)],
                                         en[:, ts(i, R)], start=True, stop=True)
                    rot_copy(wp[:, ts(og, 512)], pO[:], seq="vss")

        def attn_h_restore(ybf_pool, h2_pool):
            with tc.tile_pool(name="ah", bufs=4) as ah, \
                 tc.tile_pool(name="pmm", bufs=2, space="PSUM") as pmm, \
                 tc.tile_pool(name="pvt", bufs=2, space="PSUM") as pvt:
                for og in range(32):
                    pS = pmm.tile([R, 512], F32, tag="ps")
                    for i in range(4):
                        o = og * 4 + i
                        nc.tensor.matmul(pS[:, ts(i, R)],
                                         qkvs[:, OK + o * R:OK + (o + 1) * R],
                                         qkvs[:, OQ + o * R:OQ + (o + 1) * R],
                                         start=True, stop=True)
                    e4 = ah.tile([R, 512], BF, tag="e4")
                    nc.scalar.activation(e4[:], pS[:], AF.Exp)
                    pvT = pvt.tile([R, 512], BF, tag="tp")
                    for i in range(4):
                        o = og * 4 + i
                        nc.tensor.transpose(pvT[:, ts(i, R)],
                                            qkvs[:, OV + o * R:OV + (o + 1) * R],
                                            id_sb[:])
                    vts = ah.tile([R, 512], BF, tag="vts")
                    nc.vector.tensor_copy(vts[:], pvT[:])
                    # column sums: reuse pS (exp already consumed it)
                    for i in range(4):
                        nc.tensor.matmul(pS[:, i:i + 1], e4[:, ts(i, R)],
                                         ones_sb[:], start=True, stop=True)
                    sT = ah.tile([R, 4], F32, tag="sT")
                    nc.vector.tensor_copy(sT[:], pS[:, 0:4])
                    pO = pmm.tile([R, 512], F32, tag="po")
                    for i in range(4):
                        nc.tensor.matmul(pO[:, ts(i, R)], e4[:, ts(i, R)],
                                         vts[:, ts(i, R)], start=True,
                                         stop=True)
                    pof = ah.tile([R, 512], F32, tag="pof")
                    rot_copy(pof[:], pO[:], seq="sv")
                    h2 = h2_pool.tile([R, 512], BF, tag="h2")
                    for i in range(4):
                        nc.gpsimd.normalize_recip(h2[:, ts(i, R)],
                                                  pof[:, ts(i, R)],
                                                  sT[:, i:i + 1])
                    # restore: y chunk for this og (bo added on host)
                    for half in range(2):
                        pY = pmm.tile([R, 512], F32, tag="py")
                        nc.tensor.matmul(pY[:], woT_sb[:, ts(half, R)],
                                         h2[:], start=True, stop=True)
                        yt = ybf_pool.tile([R, 512], BF, tag="yt")
                        rot_copy(yt[:], pY[:], seq="sv")
                        nc.sync.dma_start(y[half * R:(half + 1) * R,
                                            ts(og, 512)], yt[:])

        # ---------- Stage A + width qkv (zp scoped) ----------
        with tc.tile_pool(name="pAB", bufs=1) as pAB:
            zp = pAB.tile([R, HW], BF, tag="zp")  # [h, w*128+r]
            with tc.tile_pool(name="xin", bufs=4) as xin, \
                 tc.tile_pool(name="pA", bufs=3, space="PSUM") as pA:
                for ch in range(8):          # 16 w-blocks per chunk
                    xa = xin.tile([R, 2048], BF, tag="xa")
                    nc.sync.dma_start(xa[:], xT[0:R, ts(ch, 2048)])
                    xb = xin.tile([R, 2048], BF, tag="xb")
                    nc.sync.dma_start(xb[:], xT[R:CIN, ts(ch, 2048)])
                    for wg in range(4):      # 4 w per psum tile
                        pa = pA.tile([R, 512], F32, tag="pa")
                        for i in range(4):
                            wl = wg * 4 + i  # w within chunk
                            nc.tensor.matmul(pa[:, ts(i, R)],
                                             xa[:, ts(wl, R)], wrT_a[:],
                                             start=True, stop=False)
                            nc.tensor.matmul(pa[:, ts(i, R)],
                                             xb[:, ts(wl, R)], wrT_b[:],
                                             start=False, stop=True)
                        g = ch * 4 + wg
                        nc.vector.tensor_tensor(zp[:, ts(g, 512)], pa[:],
                                                brp_sb[:], op=ALU.add)

            qkv_phase(zp, wqkvw_sb, corrw_sb)

        # ---------- width attention -> wp, height qkv ----------
        with tc.tile_pool(name="pW", bufs=1) as pW:
            wp = pW.tile([R, HW], BF, tag="wp")  # [w, o*128+r]
            attn_w_phase(wp)
            qkv_phase(wp, wqkvh_sb, corrh_sb)

        # ---------- height attention + restore (streamed) ----------
        with tc.tile_pool(name="h2p", bufs=4) as h2p, \
             tc.tile_pool(name="yout", bufs=6) as yp:
            attn_h_restore(yp, h2p)

    nc.compile()
    return nc


def _get_nc():
    if "nc" not in _CACHE:
        _CACHE["nc"] = _build()
    return _CACHE["nc"]


def _host_inputs(inputs):
    bf16 = ml_dtypes.bfloat16
    f32 = np.float32
    g = {k: np.asarray(v) for k, v in inputs.items()}
    Wr, br = g["Wr"], g["br"]
    Wqw, bqw, Wkw, Wvw, bvw = g["Wqw"], g["bqw"], g["Wkw"], g["Wvw"], g["bvw"]
    Wqh, bqh, Wkh, Wvh, bvh = g["Wqh"], g["bqh"], g["Wkh"], g["Wvh"], g["bvh"]
    Wo = g["Wo"]

    def tile_row(v):
        return np.tile(np.asarray(v)[None, :], (R, 1))

    com = dict(
        wrT=np.ascontiguousarray(Wr.T).astype(bf16),
        brp_row=np.ascontiguousarray(np.tile(np.asarray(br)[None, :], (1, 4))
                                     ).astype(bf16),
        ones_row=np.ones((1, R)).astype(bf16),
        wqkv_w=np.ascontiguousarray(
            np.concatenate([Wqw.T, Wkw.T, Wvw.T], 1)).astype(bf16),
        corr_w=np.ascontiguousarray(np.concatenate(
            [tile_row(bqw), tile_row(bvw)], 1)).astype(bf16),
        wqkv_h=np.ascontiguousarray(
            np.concatenate([Wqh.T, Wkh.T, Wvh.T], 1)).astype(bf16),
        corr_h=np.ascontiguousarray(np.concatenate(
            [tile_row(bqh), tile_row(bvh)], 1)).astype(bf16),
        woT=np.ascontiguousarray(Wo.T).astype(bf16),
        ident=np.eye(R).astype(bf16),
        ones=np.ones((R, 1)).astype(bf16),
    )
    xs = np.asarray(g["x"])
    in_maps = []
    for b in range(NCORES):
        m = dict(com)
        m["xT"] = np.ascontiguousarray(
            xs[b].transpose(0, 2, 1).reshape(CIN, HW)).astype(bf16)
        in_maps.append(m)
    return in_maps


def run(inputs, trace=False, **kw):
    try:
        from concourse.bass_utils import run_bass_kernel_spmd
    except ImportError:
        import sys
        for p in ("/opt/trn_rl_repo", "/root/.axon_site/_ro/trn_rl_repo"):
            if p not in sys.path:
                sys.path.append(p)
        from concourse.bass_utils import run_bass_kernel_spmd
    nc = _get_nc()
    in_maps = _host_inputs(inputs)
    res = run_bass_kernel_spmd(nc, in_maps, list(range(NCORES)),
                               trace=trace, **kw)
    bo = np.asarray(inputs["bo"], dtype=np.float32)
    out = np.stack([np.asarray(res.results[b]["y"], dtype=np.float32)
                    .reshape(COUT, R, R).transpose(0, 2, 1)
                    for b in range(NCORES)], 0)
    out += bo[None, :, None, None]
    return out.astype(np.float32), res


def kernel(**inputs):
    out, _ = run(inputs, trace=False)
    return out


# revision 36
# speedup vs baseline: 1.1795x; 1.0455x over previous
"""CSAA (criss-cross axial attention) Trainium2 kernel, v2.

Sharding: pure data parallel — batch element b -> NeuronCore b (B=8 on 8 cores).

Per-core pipeline (R=128, CIN=COUT=256, H=W=128, HW=16384), per batch element:
  xT [c, (w,h)] bf16 (host-transposed)
    --stage A (per-w stationary mm)-->  Zp [h, (w,r)] bf16          (+br)
  width branch:  qkv (per-r stationary mm) -> qkvs [w, s,o,c]       (+corr_w)
                 S^T-form attention -> Wp [w, (h,r)] bf16
  height branch: qkv -> qkvs [h, s,o',c]                            (+corr_h)
                 S^T-form attention -> H2 [r, (w,h)] streamed
  restore:       Y [co, (w,h)] bf16 -> host f32 + transpose + bo

Attention (S^T form, softmax over the PARTITION axis):
  S^T[u,r] = matmul(lhsT=k_blk, rhs=q_blk); E = exp(S^T) bf16
  width:  sums = gpsimd.partition_all_reduce(E) (replicated f32);
          OutT[w,r] = matmul(lhsT=V^T_blk, rhs=E_blk); out = OutT / sums
  height: sums_T[r,1] = matmul(lhsT=E_blk, rhs=ones); rcT = 1/sums_T;
          Out[r,h] = matmul(lhsT=E_blk, rhs=V^T_blk); out = Out * rcT[r]

Bias exactness: q-bias via corr tile (affects softmax); k-bias dropped
(u-independent shift, softmax-invariant in S^T form); v-bias added at the
qkv copy via corr v-section (rides through attention exactly since softmax
rows sum to 1); bo added on host.
"""

import numpy as np
from contextlib import ExitStack

import ml_dtypes

R = 128
CIN = 256
COUT = 256
HW = R * R
NCORES = 8
LINEARIZE = False
_CACHE = {}


def _build():
    try:
        import concourse.bass as bass
    except ImportError:
        import sys
        for p in ("/opt/trn_rl_repo", "/root/.axon_site/_ro/trn_rl_repo"):
            if p not in sys.path:
                sys.path.append(p)
        import concourse.bass as bass
    import concourse.tile as tile
    from concourse import bacc, mybir, bass_isa

    BF = mybir.dt.bfloat16
    F32 = mybir.dt.float32
    AF = mybir.ActivationFunctionType
    ALU = mybir.AluOpType
    RED = bass_isa.ReduceOp
    ts = bass.ts

    nc = bacc.Bacc("TRN2", target_bir_lowering=False, debug=False)

    def din(name, shape, dt):
        return nc.dram_tensor(name, shape, dt, kind="ExternalInput").ap()

    xT = din("xT", [CIN, HW], BF)        # [c, w*128+h]
    wrT = din("wrT", [CIN, R], BF)
    brp = din("brp", [R, 512], F32)      # br[r] tiled x4, replicated rows
    wqkv_w = din("wqkv_w", [R, 3 * R], BF)   # [WqwT|WkwT|WvwT]
    corr_w = din("corr_w", [R, 2 * R], BF)   # [bqw tile | bvw tile]
    wqkv_h = din("wqkv_h", [R, 3 * R], BF)
    corr_h = din("corr_h", [R, 2 * R], BF)   # [bqh tile | bvh tile]
    woT = din("woT", [R, COUT], BF)
    ident = din("ident", [R, R], BF)
    ones = din("ones", [R, 1], BF)
    y = nc.dram_tensor("y", [COUT, HW], BF, kind="ExternalOutput").ap()

    OQ, OK, OV = 0, HW, 2 * HW

    with tile.TileContext(nc, linearize=LINEARIZE) as tc, ExitStack() as ctx:
        const = ctx.enter_context(tc.tile_pool(name="const", bufs=1))

        _cn = [0]

        def cload(ap, dt):
            _cn[0] += 1
            t = const.tile(list(ap.shape), dt, tag=f"c{_cn[0]}_{ap.tensor.name}")
            nc.sync.dma_start(t[:], ap)
            return t

        wrT_a = cload(wrT[0:R, :], BF)
        wrT_b = cload(wrT[R:CIN, :], BF)
        brp_sb = cload(brp, F32)
        wqkvw_sb = cload(wqkv_w, BF)
        corrw_sb = cload(corr_w, BF)
        wqkvh_sb = cload(wqkv_h, BF)
        corrh_sb = cload(corr_h, BF)
        woT_sb = cload(woT, BF)
        id_sb = cload(ident, BF)
        ones_sb = cload(ones, BF)

        _rc = [0]

        def rot_copy(dst, src, seq="vs"):
            e = seq[_rc[0] % len(seq)]
            _rc[0] += 1
            if e == "v":
                nc.vector.tensor_copy(dst, src)
            else:
                nc.scalar.activation(dst, src, AF.Identity)

        qkvp = ctx.enter_context(tc.tile_pool(name="qkvp", bufs=1))
        qkvs = qkvp.tile([R, 3 * HW], BF, tag="qkvs")  # [q|k|v], [p, o*128+c]
        qkvs_c = qkvs[:].rearrange("p (s o c) -> p c s o", s=3, c=R)

        def qkv_phase(src, wq_sb, corr_sb):
            # src [p, (t, c)] : contract over p, per-c stationary slice
            src_v = src[:].rearrange("p (t c) -> p c t", c=R)
            corr_q = corr_sb[:, 0:R][:, None, :].broadcast_to([R, 4, R])
            corr_v = corr_sb[:, R:2 * R][:, None, :].broadcast_to([R, 4, R])
            with tc.tile_pool(name="pQ", bufs=2, space="PSUM") as pQ:
                for cg in range(32):
                    pq = pQ.tile([R, 2048], F32, tag="pq")
                    for i in range(4):
                        c = cg * 4 + i
                        nc.tensor.matmul(pq[:, i * 512:i * 512 + 384],
                                         src_v[:, c, :], wq_sb[:],
                                         start=True, stop=True)
                    pq_v = pq[:].rearrange("p (c x) -> p c x", c=4)
                    pq_v = pq_v[:, :, 0:384].rearrange("p c (s o) -> p c s o",
                                                       s=3)
                    cg4 = slice(cg * 4, cg * 4 + 4)
                    dq, dk, dv = (qkvs_c[:, cg4, s, :] for s in range(3))
                    sq, sk, sv = (pq_v[:, :, s, :] for s in range(3))
                    # q: DVE psum-add; v: ACT copy + Pool in-place bf16 add;
                    # k: pure copy alternating DVE/ACT
                    nc.vector.tensor_tensor(dq, sq, corr_q, op=ALU.add)
                    nc.scalar.activation(dv, sv, AF.Identity)
                    nc.gpsimd.tensor_tensor(dv, dv, corr_v, op=ALU.add)
                    if cg % 2 == 0:
                        nc.vector.tensor_copy(dk, sk)
                    else:
                        nc.scalar.activation(dk, sk, AF.Identity)

        def attn_w_phase(wp):
            with tc.tile_pool(name="aw", bufs=4) as aw, \
                 tc.tile_pool(name="pmm", bufs=3, space="PSUM") as pmm, \
                 tc.tile_pool(name="pvt", bufs=2, space="PSUM") as pvt:
                for og in range(32):
                    pS = pmm.tile([R, 512], F32, tag="ps")
                    for i in range(4):
                        o = og * 4 + i
                        nc.tensor.matmul(pS[:, ts(i, R)],
                                         qkvs[:, OK + o * R:OK + (o + 1) * R],
                                         qkvs[:, OQ + o * R:OQ + (o + 1) * R],
                                         start=True, stop=True)
                    e4 = aw.tile([R, 512], BF, tag="e4")
                    nc.scalar.activation(e4[:], pS[:], AF.Exp)
                    sums = aw.tile([R, 512], F32, tag="sums")
                    nc.gpsimd.partition_all_reduce(sums[:], e4[:], R, RED.add)
                    rc = aw.tile([R, 512], BF, tag="rc")
                    with nc.allow_low_precision(reason="softmax recip bf16"):
                        nc.vector.reciprocal(rc[:], sums[:])
                    en = aw.tile([R, 512], BF, tag="en")
                    nc.vector.tensor_tensor(en[:], e4[:], rc[:], op=ALU.mult)
                    pvT = pvt.tile([R, 512], BF, tag="tp")
                    for i in range(4):
                        o = og * 4 + i
                        nc.tensor.transpose(pvT[:, ts(i, R)],
                                            qkvs[:, OV + o * R:OV + (o + 1) * R],
                                            id_sb[:])
                    vts = aw.tile([R, 512], BF, tag="vts")
                    nc.vector.tensor_copy(vts[:], pvT[:])
                    pO = pmm.tile([R, 512], F32, tag="po")
                    for i in range(4):
                        nc.tensor.matmul(pO[:, ts(i, R)], vts[:, ts(i, R)],
                                         en[:, ts(i, R)], start=True, stop=True)
                    nc.scalar.activation(wp[:, ts(og, 512)], pO[:],
                                         AF.Identity)

        def attn_h_restore(ybf_pool, h2_pool):
            with tc.tile_pool(name="ah", bufs=4) as ah, \
                 tc.tile_pool(name="pmm", bufs=2, space="PSUM") as pmm, \
                 tc.tile_pool(name="pvt", bufs=2, space="PSUM") as pvt:
                for og in range(32):
                    pS = pmm.tile([R, 512], F32, tag="ps")
                    for i in range(4):
                        o = og * 4 + i
                        nc.tensor.matmul(pS[:, ts(i, R)],
                                         qkvs[:, OK + o * R:OK + (o + 1) * R],
                                         qkvs[:, OQ + o * R:OQ + (o + 1) * R],
                                         start=True, stop=True)
                    e4 = ah.tile([R, 512], BF, tag="e4")
                    nc.scalar.activation(e4[:], pS[:], AF.Exp)
                    pvT = pvt.tile([R, 512], BF, tag="tp")
                    for i in range(4):
                        o = og * 4 + i
                        nc.tensor.transpose(pvT[:, ts(i, R)],
                                            qkvs[:, OV + o * R:OV + (o + 1) * R],
                                            id_sb[:])
                    vts = ah.tile([R, 512], BF, tag="vts")
                    nc.vector.tensor_copy(vts[:], pvT[:])
                    # column sums: reuse pS (exp already consumed it)
                    for i in range(4):
                        nc.tensor.matmul(pS[:, i:i + 1], e4[:, ts(i, R)],
                                         ones_sb[:], start=True, stop=True)
                    sT = ah.tile([R, 4], F32, tag="sT")
                    nc.vector.tensor_copy(sT[:], pS[:, 0:4])
                    pO = pmm.tile([R, 512], F32, tag="po")
                    for i in range(4):
                        nc.tensor.matmul(pO[:, ts(i, R)], e4[:, ts(i, R)],
                                         vts[:, ts(i, R)], start=True,
                                         stop=True)
                    pof = ah.tile([R, 512], F32, tag="pof")
                    rot_copy(pof[:], pO[:], seq="sv")
                    h2 = h2_pool.tile([R, 512], BF, tag="h2")
                    for i in range(4):
                        nc.gpsimd.normalize_recip(h2[:, ts(i, R)],
                                                  pof[:, ts(i, R)],
                                                  sT[:, i:i + 1])
                    # restore: y chunk for this og (bo added on host)
                    for half in range(2):
                        pY = pmm.tile([R, 512], F32, tag="py")
                        nc.tensor.matmul(pY[:], woT_sb[:, ts(half, R)],
                                         h2[:], start=True, stop=True)
                        yt = ybf_pool.tile([R, 512], BF, tag="yt")
                        if half == 0:
                            nc.scalar.activation(yt[:], pY[:], AF.Identity)
                        else:
                            nc.vector.tensor_copy(yt[:], pY[:])
                        nc.sync.dma_start(y[half * R:(half + 1) * R,
                                            ts(og, 512)], yt[:])

        # ---------- Stage A + width qkv (zp scoped) ----------
        with tc.tile_pool(name="pAB", bufs=1) as pAB:
            zp = pAB.tile([R, HW], BF, tag="zp")  # [h, w*128+r]
            with tc.tile_pool(name="xin", bufs=4) as xin, \
                 tc.tile_pool(name="pA", bufs=3, space="PSUM") as pA:
                for ch in range(8):          # 16 w-blocks per chunk
                    xa = xin.tile([R, 2048], BF, tag="xa")
                    nc.sync.dma_start(xa[:], xT[0:R, ts(ch, 2048)])
                    xb = xin.tile([R, 2048], BF, tag="xb")
                    nc.sync.dma_start(xb[:], xT[R:CIN, ts(ch, 2048)])
                    for wg in range(4):      # 4 w per psum tile
                        pa = pA.tile([R, 512], F32, tag="pa")
                        for i in range(4):
                            wl = wg * 4 + i  # w within chunk
                            nc.tensor.matmul(pa[:, ts(i, R)],
                                             xa[:, ts(wl, R)], wrT_a[:],
                                             start=True, stop=False)
                            nc.tensor.matmul(pa[:, ts(i, R)],
                                             xb[:, ts(wl, R)], wrT_b[:],
                                             start=False, stop=True)
                        g = ch * 4 + wg
                        nc.vector.tensor_tensor(zp[:, ts(g, 512)], pa[:],
                                                brp_sb[:], op=ALU.add)

            qkv_phase(zp, wqkvw_sb, corrw_sb)

        # ---------- width attention -> wp, height qkv ----------
        with tc.tile_pool(name="pW", bufs=1) as pW:
            wp = pW.tile([R, HW], BF, tag="wp")  # [w, o*128+r]
            attn_w_phase(wp)
            qkv_phase(wp, wqkvh_sb, corrh_sb)

        # ---------- height attention + restore (streamed) ----------
        with tc.tile_pool(name="h2p", bufs=4) as h2p, \
             tc.tile_pool(name="yout", bufs=6) as yp:
            attn_h_restore(yp, h2p)

    nc.compile()
    return nc


def _get_nc():
    if "nc" not in _CACHE:
        _CACHE["nc"] = _build()
    return _CACHE["nc"]


def _host_inputs(inputs):
    bf16 = ml_dtypes.bfloat16
    f32 = np.float32
    g = {k: np.asarray(v) for k, v in inputs.items()}
    Wr, br = g["Wr"], g["br"]
    Wqw, bqw, Wkw, Wvw, bvw = g["Wqw"], g["bqw"], g["Wkw"], g["Wvw"], g["bvw"]
    Wqh, bqh, Wkh, Wvh, bvh = g["Wqh"], g["bqh"], g["Wkh"], g["Wvh"], g["bvh"]
    Wo = g["Wo"]

    def tile_row(v):
        return np.tile(np.asarray(v)[None, :], (R, 1))

    com = dict(
        wrT=np.ascontiguousarray(Wr.T).astype(bf16),
        brp=np.ascontiguousarray(np.tile(np.asarray(br)[None, :], (R, 4))
                                 ).astype(f32),
        wqkv_w=np.ascontiguousarray(
            np.concatenate([Wqw.T, Wkw.T, Wvw.T], 1)).astype(bf16),
        corr_w=np.ascontiguousarray(np.concatenate(
            [tile_row(bqw), tile_row(bvw)], 1)).astype(bf16),
        wqkv_h=np.ascontiguousarray(
            np.concatenate([Wqh.T, Wkh.T, Wvh.T], 1)).astype(bf16),
        corr_h=np.ascontiguousarray(np.concatenate(
            [tile_row(bqh), tile_row(bvh)], 1)).astype(bf16),
        woT=np.ascontiguousarray(Wo.T).astype(bf16),
        ident=np.eye(R).astype(bf16),
        ones=np.ones((R, 1)).astype(bf16),
    )
    xs = np.asarray(g["x"])
    in_maps = []
    for b in range(NCORES):
        m = dict(com)
        m["xT"] = np.ascontiguousarray(
            xs[b].transpose(0, 2, 1).reshape(CIN, HW)).astype(bf16)
        in_maps.append(m)
    return in_maps


def run(inputs, trace=False, **kw):
    try:
        from concourse.bass_utils import run_bass_kernel_spmd
    except ImportError:
        import sys
        for p in ("/opt/trn_rl_repo", "/root/.axon_site/_ro/trn_rl_repo"):
            if p not in sys.path:
                sys.path.append(p)
        from concourse.bass_utils import run_bass_kernel_spmd
    nc = _get_nc()
    in_maps = _host_inputs(inputs)
    res = run_bass_kernel_spmd(nc, in_maps, list(range(NCORES)),
                               trace=trace, **kw)
    bo = np.asarray(inputs["bo"], dtype=np.float32)
    out = np.stack([np.asarray(res.results[b]["y"], dtype=np.float32)
                    .reshape(COUT, R, R).transpose(0, 2, 1)
                    for b in range(NCORES)], 0)
    out += bo[None, :, None, None]
    return out.astype(np.float32), res


def kernel(**inputs):
    out, _ = run(inputs, trace=False)
    return out
